# revision 3
# baseline (speedup 1.0000x reference)
"""Adaptive-softmax CE loss on 8 TRN2 NeuronCores.

Strategy v2: the CE is masked per cluster, so tail logsumexps are only
needed for tokens IN that cluster (~16% for tail0, ~80% for tail1).
  - Head (2002-wide lse, all 4096 tokens): data-parallel, 512 tokens/core.
  - Tails: host compacts cluster tokens (T0~633 -> 640, T1~3293 -> 3328),
    then TENSOR-PARALLEL vocab split: every core computes h for ALL
    compacted tail tokens (PE has slack) but only its 1/8 vocab slice
    (1000 of 8000, 5000 of 40000).  Host sums the 8 per-core sum-exp
    partials per token (sharded logsumexp) - no collectives.
This cuts ScalarE exp work from 25.6M to ~18.4M elems/core (the hard
floor: exp runs only on ScalarE at 128 lanes * 1.2 GHz), and shrinks
per-core weight traffic enough that ALL weights are SBUF-resident
(w2t1 slice = 1.25 MB) - no streaming.

Per-tile reduction policy: ACT accum_out for t1 tile0 + head + t0
(read-accum costs ~290ns/instr), DVE tensor_reduce over bf16 exp tiles
for t1 tiles 1-2 (DVE has slack).  Predicted busy: ACT ~146us,
DVE ~130us, PE ~110us.

Numerics as baseline: fp8 DoubleRow matmuls with x64-scaled weights,
undone for free via exp(x/64); label dots from the same fp8 h so
quantization noise partially cancels in lse - dot; host finishes in
float64 (log, masks, average).
"""

import numpy as np
import ml_dtypes

CUTOFF = [2000, 10000, 50000]
N_TOK = 4096
D = 1024
N_CORES = 8
TOK_PER_CORE = N_TOK // N_CORES          # 512
N_BLK = TOK_PER_CORE // 128              # 4 head token blocks
KX = 9                                   # augmented-input chunks (1152/128)
K0 = 8                                   # 1024/128
K1 = 2                                   # 256/128
N_HEAD = CUTOFF[0] + 2                   # 2002
V0 = CUTOFF[1] - CUTOFF[0]               # 8000
V1 = CUTOFF[2] - CUTOFF[1]               # 40000
V0S = V0 // N_CORES                      # 1000 per-core tail0 vocab slice
V1S = V1 // N_CORES                      # 5000 per-core tail1 vocab slice
WSCALE = 64.0

BF16 = ml_dtypes.bfloat16
FP8 = ml_dtypes.float8_e4m3

_cache = {}


def _subs(width):
    out, o = [], 0
    while o < width:
        out.append((o, min(512, width - o)))
        o += min(512, width - o)
    return out


def _wins(total, step=512):
    return _subs(total) if step == 512 else None


def _t1_tiles():
    # per-block vocab tiles of the 5000-wide per-core slice
    return [(0, 2048), (2048, 2048), (4096, V1S - 4096)]


def _build_nc(b0, b1):
    import concourse.bass as bass
    import concourse.bacc as bacc
    import concourse.mybir as mybir
    from concourse import tile

    t0c = b0 * 128
    t1c = b1 * 128

    dt = mybir.dt
    nc = bacc.Bacc(None)

    EXP = mybir.ActivationFunctionType.Exp
    MULT = mybir.AluOpType.mult
    ADD = mybir.AluOpType.add
    DR = mybir.MatmulPerfMode.DoubleRow
    X = mybir.AxisListType.X
    PSUM = bass.MemorySpace.PSUM

    xh8_p = nc.declare_dram_parameter("xh8", [K0, 128, TOK_PER_CORE], dt.float8e4, isOutput=False)
    hwt8_p = nc.declare_dram_parameter("hwt8", [K0, 128, N_HEAD], dt.float8e4, isOutput=False)
    hbias_p = nc.declare_dram_parameter("hbias", [1, N_HEAD], dt.bfloat16, isOutput=False)
    xt_p = nc.declare_dram_parameter("xt", [KX, 128, TOK_PER_CORE], dt.bfloat16, isOutput=False)
    gh_p = nc.declare_dram_parameter("gh", [KX, 128, TOK_PER_CORE], dt.bfloat16, isOutput=False)
    x08_p = nc.declare_dram_parameter("x08", [K0, 128, t0c], dt.float8e4, isOutput=False)
    x18_p = nc.declare_dram_parameter("x18", [K0, 128, t1c], dt.float8e4, isOutput=False)
    w1t0_p = nc.declare_dram_parameter("w1t0", [K0, 128, 1024], dt.float8e4, isOutput=False)
    w1t1_p = nc.declare_dram_parameter("w1t1", [K0, 128, 256], dt.float8e4, isOutput=False)
    w2t0_p = nc.declare_dram_parameter("w2t0", [K0, 128, V0S], dt.float8e4, isOutput=False)
    w2t1_p = nc.declare_dram_parameter("w2t1", [K1, 128, V1S], dt.float8e4, isOutput=False)
    g0_p = nc.declare_dram_parameter("g0", [K0, 128, t0c], dt.bfloat16, isOutput=False)
    g1_p = nc.declare_dram_parameter("g1", [K1, 128, t1c], dt.bfloat16, isOutput=False)

    ncols = 2 * N_BLK + b0 + 3 * b1
    nll = TOK_PER_CORE + t0c + t1c
    out_s_p = nc.declare_dram_parameter("out_s", [128, ncols], dt.float32, isOutput=True)
    out_ll_p = nc.declare_dram_parameter("out_ll", [1, nll], dt.float32, isOutput=True)

    cols = []

    def dma3(dst, src, sl=None):
        if sl is None:
            nc.sync.dma_start(dst[:], src.rearrange("c p t -> p c t"))
        else:
            nc.sync.dma_start(dst[:, :, sl], src[:, :, sl].rearrange("c p t -> p c t"))

    with tile.TileContext(nc) as tc:
        with (
            tc.tile_pool(name="res", bufs=1) as res,
            tc.tile_pool(name="prs", bufs=4) as prs,
            tc.tile_pool(name="es", bufs=4) as es,
        ):
            xh8 = res.tile([128, K0, TOK_PER_CORE], dt.float8e4, tag="xh8")
            hwt8 = res.tile([128, K0, N_HEAD], dt.float8e4, tag="hwt8")
            hbias = res.tile([1, N_HEAD], dt.bfloat16, tag="hbias")
            xt = res.tile([128, KX, TOK_PER_CORE], dt.bfloat16, tag="xt")
            gh = res.tile([128, KX, TOK_PER_CORE], dt.bfloat16, tag="gh")
            x08 = res.tile([128, K0, t0c], dt.float8e4, tag="x08")
            x18 = res.tile([128, K0, t1c], dt.float8e4, tag="x18")
            w1t0 = res.tile([128, K0, 1024], dt.float8e4, tag="w1t0")
            w1t1 = res.tile([128, K0, 256], dt.float8e4, tag="w1t1")
            w2t0 = res.tile([128, K0, V0S], dt.float8e4, tag="w2t0")
            w2t1 = res.tile([128, K1, V1S], dt.float8e4, tag="w2t1")
            g0 = res.tile([128, K0, t0c], dt.bfloat16, tag="g0")
            g1 = res.tile([128, K1, t1c], dt.bfloat16, tag="g1")
            ht0_8 = res.tile([128, K0, t0c], dt.float8e4, tag="ht0_8")
            ht1_8 = res.tile([128, K1, t1c], dt.float8e4, tag="ht1_8")
            sall = res.tile([128, ncols], dt.float32, tag="sall")
            ll = res.tile([1, nll], dt.float32, tag="ll")
            ones = res.tile([128, 1], dt.bfloat16, tag="ones")
            ones1 = res.tile([1, 128], dt.bfloat16, tag="ones1")

            nc.gpsimd.memset(ones[:], 1.0)
            nc.gpsimd.memset(ones1[:], 1.0)

            with tc.tile_pool(name="pc", bufs=2, space=PSUM) as pcp:

                def exp_reduce(pc, w, mode, kind, b):
                    col = len(cols)
                    cols.append((kind, b))
                    if mode == "acc":
                        nc.scalar.activation(
                            pc[:, :w], pc[:, :w], EXP,
                            scale=1.0 / WSCALE, accum_out=sall[:, col:col + 1],
                        )
                    else:
                        et = es.tile([128, 2048], dt.bfloat16, tag="e")
                        nc.scalar.activation(et[:, :w], pc[:, :w], EXP,
                                             scale=1.0 / WSCALE)
                        nc.vector.tensor_reduce(sall[:, col:col + 1], et[:, :w],
                                                axis=X, op=ADD)

                def mm_group(pc, sl, btok, kk, lhs3, rhs3, bias=False, rbase=0):
                    rsl = slice(rbase + sl.start, rbase + sl.stop)
                    for c in range(kk // 2):
                        nc.tensor.matmul(
                            pc[:, sl],
                            lhsT=lhs3[:, 2 * c:2 * c + 2, btok * 128:(btok + 1) * 128],
                            rhs=rhs3[:, 2 * c:2 * c + 2, rsl],
                            start=(c == 0),
                            stop=(c == kk // 2 - 1 and not bias),
                            perf_mode=DR,
                        )
                    if bias:
                        nc.tensor.matmul(pc[:, sl], lhsT=ones1[:],
                                         rhs=hbias[0:1, rsl], start=False, stop=True)

                def emit_head(b, hf):
                    base = hf * 1024
                    width = min(1024, N_HEAD - base)
                    pc = pcp.tile([128, 2048], dt.float32, tag="pc")
                    for off, w in _subs(width):
                        mm_group(pc, slice(off, off + w), b, K0, xh8, hwt8,
                                 bias=True, rbase=base)
                    exp_reduce(pc, width, "acc", "h", b)

                def emit_t0(b):
                    pc = pcp.tile([128, 2048], dt.float32, tag="pc")
                    for off, w in _subs(V0S):
                        mm_group(pc, slice(off, off + w), b, K0, ht0_8, w2t0)
                    exp_reduce(pc, V0S, "acc", "t0", b)

                def emit_t1(b, j):
                    off0, width = _t1_tiles()[j]
                    pc = pcp.tile([128, 2048], dt.float32, tag="pc")
                    for off, w in _subs(width):
                        mm_group(pc, slice(off, off + w), b, K1, ht1_8, w2t1,
                                 rbase=off0)
                    exp_reduce(pc, width, "acc" if j == 0 else "dve", "t1", b)

                def emit_a(w1t, x8, ht_8, ms, wsl):
                    # phase-A h tiles: pack len(ms) windows of wlen tokens
                    # into one PSUM tile; DVE rescales to fp8 SBUF
                    wlen = wsl.stop - wsl.start
                    pt = pcp.tile([128, 2048], dt.float32, tag="pc")
                    for i, m in enumerate(ms):
                        psl = slice(i * wlen, (i + 1) * wlen)
                        for c in range(K0 // 2):
                            nc.tensor.matmul(
                                pt[:, psl],
                                lhsT=w1t[:, 2 * c:2 * c + 2, m * 128:(m + 1) * 128],
                                rhs=x8[:, 2 * c:2 * c + 2, wsl],
                                start=(c == 0), stop=(c == K0 // 2 - 1),
                                perf_mode=DR,
                            )
                    for i, m in enumerate(ms):
                        psl = slice(i * wlen, (i + 1) * wlen)
                        nc.vector.tensor_scalar_mul(ht_8[:, m, wsl], pt[:, psl],
                                                    1.0 / WSCALE)

                def emit_lab(chunk_list, wsl, llbase):
                    # label dots: per 512-token window, elementwise mul per
                    # k-chunk (DVE) + ones-matmul partition reduce (PE)
                    wlen = wsl.stop - wsl.start
                    pl = pcp.tile([128, 2048], dt.float32, tag="pc")
                    pll = pl[0:1, :wlen]
                    n = len(chunk_list)
                    for i, (lhs, rhs) in enumerate(chunk_list):
                        pr = prs.tile([128, 512], dt.bfloat16, tag="pr")
                        nc.vector.tensor_tensor(pr[:, :wlen], lhs[:, wsl],
                                                rhs[:, wsl], op=MULT)
                        nc.tensor.matmul(pll, lhsT=ones[:], rhs=pr[:, :wlen],
                                         start=(i == 0), stop=(i == n - 1))
                    gsl = slice(llbase + wsl.start, llbase + wsl.stop)
                    nc.vector.tensor_copy(ll[0:1, gsl], pll)

                # ---------------- emission schedule ----------------
                n1w = (t1c + 511) // 512          # phase-A t1 windows
                n0w = (t0c + 511) // 512
                hch = [(xt[:, k, :], gh[:, k, :]) for k in range(KX)]
                t0ch = [(ht0_8[:, k, :], g0[:, k, :]) for k in range(K0)]
                t1ch = [(ht1_8[:, k, :], g1[:, k, :]) for k in range(K1)]

                # startup DMAs + phase A for first t1 blocks
                dma3(w1t1, w1t1_p)
                a1w = _subs(t1c)
                dma3(x18, x18_p, slice(a1w[0][0], a1w[0][0] + a1w[0][1]))
                dma3(w2t1, w2t1_p, slice(0, 2048))
                dma3(xh8, xh8_p)
                wsl0 = slice(a1w[0][0], a1w[0][0] + a1w[0][1])
                emit_a(w1t1, x18, ht1_8, [0, 1], wsl0)
                dma3(x18, x18_p, slice(a1w[1][0], a1w[1][0] + a1w[1][1]))
                dma3(w2t1, w2t1_p, slice(2048, 4096))
                wsl1 = slice(a1w[1][0], a1w[1][0] + a1w[1][1])
                emit_a(w1t1, x18, ht1_8, [0, 1], wsl1)
                nc.sync.dma_start(hwt8[:, :, 0:1024],
                                  hwt8_p[:, :, 0:1024].rearrange("c p t -> p c t"))
                dma3(w2t1, w2t1_p, slice(4096, V1S))
                nc.sync.dma_start(hbias[:], hbias_p[:])

                # extras sprinkled between t1 blocks; one list per block slot
                extras = [[] for _ in range(b1)]
                extras[0] = [("hd", 0, 0)]
                extras[1] = [("a1", 2), ("dma", "hw1")]
                extras[2] = [("hd", 0, 1), ("dma", "x08")]
                extras[3] = [("a1", 3), ("dma", "w1t0")]
                extras[4] = [("hd", 1, 0), ("dma", "w2t0")]
                extras[5] = [("a1", 4), ("hd", 1, 1)]
                extras[6] = [("a0", 0, [0, 1, 2, 3]), ("dma", "xt")]
                extras[7] = [("a1", 5), ("a0", 0, [4, 5, 6, 7])]
                extras[8] = [("hd", 2, 0), ("dma", "gh")]
                if n0w > 1:
                    extras[9] = [("a1", 6), ("a0", 1, list(range(8)))]
                else:
                    extras[9] = [("a1", 6)]
                extras[10] = [("hd", 2, 1), ("dma", "g0")]
                extras[11] = [("t0", 0), ("dma", "g1a")]
                extras[12] = [("hd", 3, 0), ("dma", "g1b")]
                extras[13] = [("t0", 1)]
                extras[14] = [("hd", 3, 1)]
                extras[15] = [("t0", 2), ("lab", "h", 0)]
                extras[16] = [("t0", 3), ("lab", "t0", 0)]
                if b0 > 4:
                    extras[17] = [("t0", 4), ("lab", "t0", 1)]
                else:
                    extras[17] = [("lab", "t0", 1)] if n0w > 1 else []
                for i in range(n1w):
                    extras[min(18 + i, b1 - 1)].append(("lab", "t1", i))
                if b1 < 18:  # tiny-cluster fallback: front-load everything
                    flat = [e for lst in extras for e in lst]
                    extras = [[] for _ in range(b1)]
                    extras[0] = flat

                def run_extra(e):
                    if e[0] == "hd":
                        emit_head(e[1], e[2])
                    elif e[0] == "a1":
                        i = e[1]
                        if i < n1w:
                            o, wl = a1w[i]
                            dma3(x18, x18_p, slice(o, o + wl))
                            emit_a(w1t1, x18, ht1_8, [0, 1], slice(o, o + wl))
                    elif e[0] == "a0":
                        wi, ms = e[1], e[2]
                        if wi < n0w:
                            o, wl = _subs(t0c)[wi]
                            emit_a(w1t0, x08, ht0_8, ms, slice(o, o + wl))
                    elif e[0] == "t0":
                        if e[1] < b0:
                            emit_t0(e[1])
                    elif e[0] == "lab":
                        g, wi = e[1], e[2]
                        if g == "h":
                            emit_lab(hch, slice(0, 512), 0)
                        elif g == "t0":
                            ws = _subs(t0c)
                            if wi < len(ws):
                                o, wl = ws[wi]
                                emit_lab(t0ch, slice(o, o + wl), TOK_PER_CORE)
                        else:
                            o, wl = a1w[wi]
                            emit_lab(t1ch, slice(o, o + wl), TOK_PER_CORE + t0c)
                    elif e[0] == "dma":
                        n = e[1]
                        if n == "hw1":
                            nc.sync.dma_start(
                                hwt8[:, :, 1024:N_HEAD],
                                hwt8_p[:, :, 1024:N_HEAD].rearrange("c p t -> p c t"))
                        elif n == "x08":
                            dma3(x08, x08_p)
                        elif n == "w1t0":
                            dma3(w1t0, w1t0_p)
                        elif n == "w2t0":
                            dma3(w2t0, w2t0_p)
                        elif n == "xt":
                            dma3(xt, xt_p)
                        elif n == "gh":
                            dma3(gh, gh_p)
                        elif n == "g0":
                            dma3(g0, g0_p)
                        elif n == "g1a":
                            dma3(g1, g1_p, slice(0, t1c // 2))
                        elif n == "g1b":
                            dma3(g1, g1_p, slice(t1c // 2, t1c))

                for b in range(b1):
                    for e in extras[b]:
                        run_extra(e)
                    for j in range(3):
                        emit_t1(b, j)

            nc.sync.dma_start(out_s_p[:], sall[:])
            nc.sync.dma_start(out_ll_p[:], ll[:])

    nc.compile()
    return nc, cols


def _prep_inputs(w_in, target, head_w, head_b, tail0_w1, tail0_w2, tail1_w1, tail1_w2):
    f32 = np.float32
    w_in = np.asarray(w_in, f32)
    target = np.asarray(target).astype(np.int64)
    head_w = np.asarray(head_w, f32)
    head_b = np.asarray(head_b, f32)
    t0w1 = np.asarray(tail0_w1, f32)
    t0w2 = np.asarray(tail0_w2, f32)
    t1w1 = np.asarray(tail1_w1, f32)
    t1w2 = np.asarray(tail1_w2, f32)

    c0, c1, c2 = CUTOFF
    mask0 = (target >= c0) & (target < c1)
    mask1 = (target >= c1) & (target < c2)
    idx0 = np.where(mask0)[0]
    idx1 = np.where(mask1)[0]
    t0n, t1n = len(idx0), len(idx1)
    b0 = max(1, -(-t0n // 128))
    b1 = max(1, -(-t1n // 128))
    t0c, t1c = b0 * 128, b1 * 128
    first_t = np.where(mask0, c0, np.where(mask1, c0 + 1, target))

    def chunks(a, k, dtype=BF16):  # [k*128, F] -> [k, 128, F]
        return np.ascontiguousarray(a.reshape(k, 128, a.shape[1])).astype(dtype)

    def padT(a, tcap):  # [T, F] -> [F, tcap]
        out = np.zeros((a.shape[1], tcap), f32)
        out[:, :a.shape[0]] = a.T
        return out

    # compacted tail inputs (same on every core)
    x08 = chunks(padT(w_in[idx0], t0c), K0, FP8)
    x18 = chunks(padT(w_in[idx1], t1c), K0, FP8)
    g0 = chunks(padT(t0w2[target[idx0] - c0], t0c), K0)
    g1 = chunks(padT(t1w2[target[idx1] - c1], t1c), K1)
    w1t0 = chunks(t0w1.T * WSCALE, K0, FP8)
    w1t1 = chunks(t1w1.T * WSCALE, K0, FP8)
    hwt8 = chunks(head_w.T * WSCALE, K0, FP8)
    hbias = (head_b[None, :] * WSCALE).astype(BF16)
    w2t0_full = (t0w2.T * WSCALE)                  # [1024, 8000]
    w2t1_full = (t1w2.T * WSCALE)                  # [256, 40000]

    gh_rows = head_w[first_t]                      # [N_TOK, 1024]
    bh = head_b[first_t]                           # [N_TOK]

    in_maps = []
    for c in range(N_CORES):
        sl = slice(c * TOK_PER_CORE, (c + 1) * TOK_PER_CORE)
        xt = np.zeros((KX * 128, TOK_PER_CORE), f32)
        xt[:D] = w_in[sl].T
        xt[D] = 1.0
        ghm = np.zeros((KX * 128, TOK_PER_CORE), f32)
        ghm[:D] = gh_rows[sl].T
        ghm[D] = bh[sl]
        in_maps.append({
            "xh8": chunks(xt[:D], K0, FP8),
            "hwt8": hwt8, "hbias": hbias,
            "xt": chunks(xt, KX),
            "gh": chunks(ghm, KX),
            "x08": x08, "x18": x18,
            "w1t0": w1t0, "w1t1": w1t1,
            "w2t0": chunks(w2t0_full[:, c * V0S:(c + 1) * V0S], K0, FP8),
            "w2t1": chunks(w2t1_full[:, c * V1S:(c + 1) * V1S], K1, FP8),
            "g0": g0, "g1": g1,
        })
    return in_maps, (b0, b1, t0n, t1n)


def _combine(results, cols, meta):
    b0, b1, t0n, t1n = meta
    t0c, t1c = b0 * 128, b1 * 128
    total = 0.0
    S0 = np.zeros((128, b0))
    S1 = np.zeros((128, b1))
    for c in range(N_CORES):
        S = results[c]["out_s"].astype(np.float64)
        Sh = np.zeros((128, N_BLK))
        for j, (k, b) in enumerate(cols):
            if k == "h":
                Sh[:, b] += S[:, j]
            elif k == "t0":
                S0[:, b] += S[:, j]
            else:
                S1[:, b] += S[:, j]
        llh = results[c]["out_ll"].astype(np.float64)[0, :TOK_PER_CORE]
        llh = llh.reshape(N_BLK, 128).T                  # [p, b]
        total += (np.log(Sh) - llh).sum()
    ll0 = results[0]["out_ll"].astype(np.float64)[0, TOK_PER_CORE:TOK_PER_CORE + t0c]
    ll1 = results[0]["out_ll"].astype(np.float64)[0, TOK_PER_CORE + t0c:TOK_PER_CORE + t0c + t1c]
    s0 = S0.T.reshape(-1)[:t0n]                          # token j = b*128 + p
    s1 = S1.T.reshape(-1)[:t1n]
    total += (np.log(s0) - ll0[:t0n]).sum()
    total += (np.log(s1) - ll1[:t1n]).sum()
    return np.float32(total / N_TOK)


def _run(inputs, trace=False):
    from concourse.bass_utils import run_bass_kernel_spmd

    in_maps, meta = _prep_inputs(**inputs)
    key = (meta[0], meta[1])
    if key not in _cache:
        _cache[key] = _build_nc(*key)
    nc, cols = _cache[key]
    res = run_bass_kernel_spmd(nc, in_maps, core_ids=list(range(N_CORES)), trace=trace)
    loss = _combine(res.results, cols, meta)
    return loss, res


def kernel(**inputs) -> np.ndarray:
    loss, _ = _run(inputs, trace=False)
    return loss


# revision 5
# speedup vs baseline: 1.0198x; 1.0198x over previous
"""Adaptive-softmax CE loss on 8 TRN2 NeuronCores.

Strategy v2: the CE is masked per cluster, so tail logsumexps are only
needed for tokens IN that cluster (~16% for tail0, ~80% for tail1).
  - Head (2002-wide lse, all 4096 tokens): data-parallel, 512 tokens/core.
  - Tails: host compacts cluster tokens (T0~633 -> 640, T1~3293 -> 3328),
    then TENSOR-PARALLEL vocab split: every core computes h for ALL
    compacted tail tokens (PE has slack) but only its 1/8 vocab slice
    (1000 of 8000, 5000 of 40000).  Host sums the 8 per-core sum-exp
    partials per token (sharded logsumexp) - no collectives.
This cuts ScalarE exp work from 25.6M to ~18.4M elems/core (the hard
floor: exp runs only on ScalarE at 128 lanes * 1.2 GHz), and shrinks
per-core weight traffic enough that ALL weights are SBUF-resident
(w2t1 slice = 1.25 MB) - no streaming.

Per-tile reduction policy: ACT accum_out for t1 tile0 + head + t0
(read-accum costs ~290ns/instr), DVE tensor_reduce over bf16 exp tiles
for t1 tiles 1-2 (DVE has slack).  Predicted busy: ACT ~146us,
DVE ~130us, PE ~110us.

Numerics as baseline: fp8 DoubleRow matmuls with x64-scaled weights,
undone for free via exp(x/64); label dots from the same fp8 h so
quantization noise partially cancels in lse - dot; host finishes in
float64 (log, masks, average).
"""

import numpy as np
import ml_dtypes

CUTOFF = [2000, 10000, 50000]
N_TOK = 4096
D = 1024
N_CORES = 8
TOK_PER_CORE = N_TOK // N_CORES          # 512
N_BLK = TOK_PER_CORE // 128              # 4 head token blocks
KX = 9                                   # augmented-input chunks (1152/128)
K0 = 8                                   # 1024/128
K1 = 2                                   # 256/128
N_HEAD = CUTOFF[0] + 2                   # 2002
V0 = CUTOFF[1] - CUTOFF[0]               # 8000
V1 = CUTOFF[2] - CUTOFF[1]               # 40000
V0S = V0 // N_CORES                      # 1000 per-core tail0 vocab slice
V1S = V1 // N_CORES                      # 5000 per-core tail1 vocab slice
WSCALE = 64.0

BF16 = ml_dtypes.bfloat16
FP8 = ml_dtypes.float8_e4m3

_cache = {}


def _subs(width):
    out, o = [], 0
    while o < width:
        out.append((o, min(512, width - o)))
        o += min(512, width - o)
    return out


def _wins(total, step=512):
    return _subs(total) if step == 512 else None


def _t1_tiles():
    # per-block vocab tiles of the 5000-wide per-core slice
    return [(0, 2048), (2048, 2048), (4096, V1S - 4096)]


def _build_nc(b0, b1):
    import concourse.bass as bass
    import concourse.bacc as bacc
    import concourse.mybir as mybir
    from concourse import tile

    t0c = b0 * 128
    t1c = b1 * 128

    dt = mybir.dt
    nc = bacc.Bacc(None)

    EXP = mybir.ActivationFunctionType.Exp
    MULT = mybir.AluOpType.mult
    ADD = mybir.AluOpType.add
    DR = mybir.MatmulPerfMode.DoubleRow
    X = mybir.AxisListType.X
    PSUM = bass.MemorySpace.PSUM

    xh8_p = nc.declare_dram_parameter("xh8", [K0, 128, TOK_PER_CORE], dt.float8e4, isOutput=False)
    hwt8_p = nc.declare_dram_parameter("hwt8", [K0, 128, N_HEAD], dt.float8e4, isOutput=False)
    hbias_p = nc.declare_dram_parameter("hbias", [1, N_HEAD], dt.bfloat16, isOutput=False)
    xt_p = nc.declare_dram_parameter("xt", [KX, 128, TOK_PER_CORE], dt.bfloat16, isOutput=False)
    gh_p = nc.declare_dram_parameter("gh", [KX, 128, TOK_PER_CORE], dt.bfloat16, isOutput=False)
    x08_p = nc.declare_dram_parameter("x08", [K0, 128, t0c], dt.float8e4, isOutput=False)
    x18_p = nc.declare_dram_parameter("x18", [K0, 128, t1c], dt.float8e4, isOutput=False)
    w1t0_p = nc.declare_dram_parameter("w1t0", [K0, 128, 1024], dt.float8e4, isOutput=False)
    w1t1_p = nc.declare_dram_parameter("w1t1", [K0, 128, 256], dt.float8e4, isOutput=False)
    w2t0_p = nc.declare_dram_parameter("w2t0", [K0, 128, V0S], dt.float8e4, isOutput=False)
    w2t1_p = nc.declare_dram_parameter("w2t1", [K1, 128, V1S], dt.float8e4, isOutput=False)
    g0_p = nc.declare_dram_parameter("g0", [K0, 128, t0c], dt.bfloat16, isOutput=False)
    g1_p = nc.declare_dram_parameter("g1", [K1, 128, t1c], dt.bfloat16, isOutput=False)

    ncols = 2 * N_BLK + b0 + 3 * b1
    nll = TOK_PER_CORE + t0c + t1c
    out_s_p = nc.declare_dram_parameter("out_s", [128, ncols], dt.float32, isOutput=True)
    out_ll_p = nc.declare_dram_parameter("out_ll", [1, nll], dt.float32, isOutput=True)

    cols = []

    def dma3(dst, src, sl=None):
        if sl is None:
            nc.sync.dma_start(dst[:], src.rearrange("c p t -> p c t"))
        else:
            nc.sync.dma_start(dst[:, :, sl], src[:, :, sl].rearrange("c p t -> p c t"))

    with tile.TileContext(nc) as tc:
        with (
            tc.tile_pool(name="res", bufs=1) as res,
            tc.tile_pool(name="prs", bufs=4) as prs,
            tc.tile_pool(name="es", bufs=4) as es,
        ):
            xh8 = res.tile([128, K0, TOK_PER_CORE], dt.float8e4, tag="xh8")
            hwt8 = res.tile([128, K0, N_HEAD], dt.float8e4, tag="hwt8")
            hbias = res.tile([1, N_HEAD], dt.bfloat16, tag="hbias")
            xt = res.tile([128, KX, TOK_PER_CORE], dt.bfloat16, tag="xt")
            gh = res.tile([128, KX, TOK_PER_CORE], dt.bfloat16, tag="gh")
            x08 = res.tile([128, K0, t0c], dt.float8e4, tag="x08")
            x18 = res.tile([128, K0, t1c], dt.float8e4, tag="x18")
            w1t0 = res.tile([128, K0, 1024], dt.float8e4, tag="w1t0")
            w1t1 = res.tile([128, K0, 256], dt.float8e4, tag="w1t1")
            w2t0 = res.tile([128, K0, V0S], dt.float8e4, tag="w2t0")
            w2t1 = res.tile([128, K1, V1S], dt.float8e4, tag="w2t1")
            g0 = res.tile([128, K0, t0c], dt.bfloat16, tag="g0")
            g1 = res.tile([128, K1, t1c], dt.bfloat16, tag="g1")
            ht0_8 = res.tile([128, K0, t0c], dt.float8e4, tag="ht0_8")
            ht1_8 = res.tile([128, K1, t1c], dt.float8e4, tag="ht1_8")
            sall = res.tile([128, ncols], dt.float32, tag="sall")
            ll = res.tile([1, nll], dt.float32, tag="ll")
            ones = res.tile([128, 1], dt.bfloat16, tag="ones")
            ones1 = res.tile([1, 128], dt.bfloat16, tag="ones1")

            nc.gpsimd.memset(ones[:], 1.0)
            nc.gpsimd.memset(ones1[:], 1.0)

            with tc.tile_pool(name="pc", bufs=2, space=PSUM) as pcp:

                def exp_reduce(pc, w, mode, kind, b):
                    col = len(cols)
                    cols.append((kind, b))
                    if mode == "acc":
                        nc.scalar.activation(
                            pc[:, :w], pc[:, :w], EXP,
                            scale=1.0 / WSCALE, accum_out=sall[:, col:col + 1],
                        )
                    else:
                        et = es.tile([128, 2048], dt.bfloat16, tag="e")
                        nc.scalar.activation(et[:, :w], pc[:, :w], EXP,
                                             scale=1.0 / WSCALE)
                        nc.vector.tensor_reduce(sall[:, col:col + 1], et[:, :w],
                                                axis=X, op=ADD)

                def mm_group(pc, sl, btok, kk, lhs3, rhs3, bias=False, rbase=0):
                    rsl = slice(rbase + sl.start, rbase + sl.stop)
                    for c in range(kk // 2):
                        nc.tensor.matmul(
                            pc[:, sl],
                            lhsT=lhs3[:, 2 * c:2 * c + 2, btok * 128:(btok + 1) * 128],
                            rhs=rhs3[:, 2 * c:2 * c + 2, rsl],
                            start=(c == 0),
                            stop=(c == kk // 2 - 1 and not bias),
                            perf_mode=DR,
                        )
                    if bias:
                        nc.tensor.matmul(pc[:, sl], lhsT=ones1[:],
                                         rhs=hbias[0:1, rsl], start=False, stop=True)

                def emit_head(b, hf):
                    base = hf * 1024
                    width = min(1024, N_HEAD - base)
                    pc = pcp.tile([128, 2048], dt.float32, tag="pc")
                    for off, w in _subs(width):
                        mm_group(pc, slice(off, off + w), b, K0, xh8, hwt8,
                                 bias=True, rbase=base)
                    exp_reduce(pc, width, "acc", "h", b)

                def emit_t0(b):
                    pc = pcp.tile([128, 2048], dt.float32, tag="pc")
                    for off, w in _subs(V0S):
                        mm_group(pc, slice(off, off + w), b, K0, ht0_8, w2t0)
                    exp_reduce(pc, V0S, "acc", "t0", b)

                def emit_t1(b, j):
                    off0, width = _t1_tiles()[j]
                    pc = pcp.tile([128, 2048], dt.float32, tag="pc")
                    for off, w in _subs(width):
                        mm_group(pc, slice(off, off + w), b, K1, ht1_8, w2t1,
                                 rbase=off0)
                    # keep-warm no-op: PE-HAM re-throttles the tensor clock
                    # to 1.2GHz after an idle activity window; a dependency-
                    # free LDWEIGHTS in each gap holds it at 2.4GHz
                    nc.tensor.ldweights(weights=xh8[:, 0:1, 0:128])
                    exp_reduce(pc, width, "acc" if j == 0 else "dve", "t1", b)

                def emit_a(w1t, x8, ht_8, ms, wsl):
                    # phase-A h tiles: pack len(ms) windows of wlen tokens
                    # into one PSUM tile; DVE rescales to fp8 SBUF
                    wlen = wsl.stop - wsl.start
                    pt = pcp.tile([128, 2048], dt.float32, tag="pc")
                    for i, m in enumerate(ms):
                        psl = slice(i * wlen, (i + 1) * wlen)
                        for c in range(K0 // 2):
                            nc.tensor.matmul(
                                pt[:, psl],
                                lhsT=w1t[:, 2 * c:2 * c + 2, m * 128:(m + 1) * 128],
                                rhs=x8[:, 2 * c:2 * c + 2, wsl],
                                start=(c == 0), stop=(c == K0 // 2 - 1),
                                perf_mode=DR,
                            )
                    for i, m in enumerate(ms):
                        psl = slice(i * wlen, (i + 1) * wlen)
                        nc.vector.tensor_scalar_mul(ht_8[:, m, wsl], pt[:, psl],
                                                    1.0 / WSCALE)

                def emit_lab(chunk_list, wsl, llbase):
                    # label dots: per 512-token window, elementwise mul per
                    # k-chunk (DVE) + ones-matmul partition reduce (PE)
                    wlen = wsl.stop - wsl.start
                    pl = pcp.tile([128, 2048], dt.float32, tag="pc")
                    pll = pl[0:1, :wlen]
                    n = len(chunk_list)
                    for i, (lhs, rhs) in enumerate(chunk_list):
                        pr = prs.tile([128, 512], dt.bfloat16, tag="pr")
                        nc.vector.tensor_tensor(pr[:, :wlen], lhs[:, wsl],
                                                rhs[:, wsl], op=MULT)
                        nc.tensor.matmul(pll, lhsT=ones[:], rhs=pr[:, :wlen],
                                         start=(i == 0), stop=(i == n - 1))
                    gsl = slice(llbase + wsl.start, llbase + wsl.stop)
                    nc.vector.tensor_copy(ll[0:1, gsl], pll)

                # ---------------- emission schedule ----------------
                n1w = (t1c + 511) // 512          # phase-A t1 windows
                n0w = (t0c + 511) // 512
                hch = [(xt[:, k, :], gh[:, k, :]) for k in range(KX)]
                t0ch = [(ht0_8[:, k, :], g0[:, k, :]) for k in range(K0)]
                t1ch = [(ht1_8[:, k, :], g1[:, k, :]) for k in range(K1)]

                # startup: head inputs first so ACT starts ~6us in, then
                # the tail1 pipeline inputs
                a1w = _subs(t1c)
                dma3(xh8, xh8_p)
                nc.sync.dma_start(hwt8[:, :, 0:1024],
                                  hwt8_p[:, :, 0:1024].rearrange("c p t -> p c t"))
                nc.sync.dma_start(hbias[:], hbias_p[:])
                dma3(w1t1, w1t1_p)
                dma3(x18, x18_p, slice(a1w[0][0], a1w[0][0] + a1w[0][1]))
                dma3(w2t1, w2t1_p, slice(0, 2048))
                emit_head(0, 0)
                dma3(x18, x18_p, slice(a1w[1][0], a1w[1][0] + a1w[1][1]))
                dma3(w2t1, w2t1_p, slice(2048, 4096))
                emit_a(w1t1, x18, ht1_8, [0, 1],
                       slice(a1w[0][0], a1w[0][0] + a1w[0][1]))
                emit_head(1, 0)
                nc.sync.dma_start(hwt8[:, :, 1024:N_HEAD],
                                  hwt8_p[:, :, 1024:N_HEAD].rearrange("c p t -> p c t"))
                dma3(w2t1, w2t1_p, slice(4096, V1S))
                emit_a(w1t1, x18, ht1_8, [0, 1],
                       slice(a1w[1][0], a1w[1][0] + a1w[1][1]))

                # extras sprinkled between t1 blocks; one list per block slot
                extras = [[] for _ in range(b1)]
                extras[0] = [("hd", 0, 1)]
                extras[1] = [("a1", 2), ("dma", "x08")]
                extras[2] = [("hd", 2, 0), ("dma", "w1t0")]
                extras[3] = [("a1", 3), ("dma", "w2t0")]
                extras[4] = [("hd", 1, 1)]
                extras[5] = [("a1", 4), ("hd", 3, 0)]
                extras[6] = [("a0", 0, [0, 1, 2, 3]), ("dma", "xt")]
                extras[7] = [("a1", 5), ("a0", 0, [4, 5, 6, 7])]
                extras[8] = [("hd", 2, 1), ("dma", "gh")]
                if n0w > 1:
                    extras[9] = [("a1", 6), ("a0", 1, list(range(8)))]
                else:
                    extras[9] = [("a1", 6)]
                extras[10] = [("hd", 3, 1), ("dma", "g0")]
                extras[11] = [("t0", 0), ("dma", "g1a")]
                extras[12] = [("dma", "g1b")]
                extras[13] = [("t0", 1)]
                extras[14] = []
                extras[15] = [("t0", 2), ("lab", "h", 0)]
                extras[16] = [("t0", 3), ("lab", "t0", 0)]
                if b0 > 4:
                    extras[17] = [("t0", 4), ("lab", "t0", 1)]
                else:
                    extras[17] = [("lab", "t0", 1)] if n0w > 1 else []
                for i in range(n1w):
                    extras[min(18 + i, b1 - 1)].append(("lab", "t1", i))
                if b1 < 18:  # tiny-cluster fallback: front-load everything
                    flat = [e for lst in extras for e in lst]
                    extras = [[] for _ in range(b1)]
                    extras[0] = flat

                def run_extra(e):
                    if e[0] == "hd":
                        emit_head(e[1], e[2])
                    elif e[0] == "a1":
                        i = e[1]
                        if i < n1w:
                            o, wl = a1w[i]
                            dma3(x18, x18_p, slice(o, o + wl))
                            emit_a(w1t1, x18, ht1_8, [0, 1], slice(o, o + wl))
                    elif e[0] == "a0":
                        wi, ms = e[1], e[2]
                        if wi < n0w:
                            o, wl = _subs(t0c)[wi]
                            emit_a(w1t0, x08, ht0_8, ms, slice(o, o + wl))
                    elif e[0] == "t0":
                        if e[1] < b0:
                            emit_t0(e[1])
                    elif e[0] == "lab":
                        g, wi = e[1], e[2]
                        if g == "h":
                            emit_lab(hch, slice(0, 512), 0)
                        elif g == "t0":
                            ws = _subs(t0c)
                            if wi < len(ws):
                                o, wl = ws[wi]
                                emit_lab(t0ch, slice(o, o + wl), TOK_PER_CORE)
                        else:
                            o, wl = a1w[wi]
                            emit_lab(t1ch, slice(o, o + wl), TOK_PER_CORE + t0c)
                    elif e[0] == "dma":
                        n = e[1]
                        if n == "hw1":
                            nc.sync.dma_start(
                                hwt8[:, :, 1024:N_HEAD],
                                hwt8_p[:, :, 1024:N_HEAD].rearrange("c p t -> p c t"))
                        elif n == "x08":
                            dma3(x08, x08_p)
                        elif n == "w1t0":
                            dma3(w1t0, w1t0_p)
                        elif n == "w2t0":
                            dma3(w2t0, w2t0_p)
                        elif n == "xt":
                            dma3(xt, xt_p)
                        elif n == "gh":
                            dma3(gh, gh_p)
                        elif n == "g0":
                            dma3(g0, g0_p)
                        elif n == "g1a":
                            dma3(g1, g1_p, slice(0, t1c // 2))
                        elif n == "g1b":
                            dma3(g1, g1_p, slice(t1c // 2, t1c))

                for b in range(b1):
                    for e in extras[b]:
                        run_extra(e)
                    for j in range(3):
                        emit_t1(b, j)

            nc.sync.dma_start(out_s_p[:], sall[:])
            nc.sync.dma_start(out_ll_p[:], ll[:])

    nc.compile()
    return nc, cols


def _prep_inputs(w_in, target, head_w, head_b, tail0_w1, tail0_w2, tail1_w1, tail1_w2):
    f32 = np.float32
    w_in = np.asarray(w_in, f32)
    target = np.asarray(target).astype(np.int64)
    head_w = np.asarray(head_w, f32)
    head_b = np.asarray(head_b, f32)
    t0w1 = np.asarray(tail0_w1, f32)
    t0w2 = np.asarray(tail0_w2, f32)
    t1w1 = np.asarray(tail1_w1, f32)
    t1w2 = np.asarray(tail1_w2, f32)

    c0, c1, c2 = CUTOFF
    mask0 = (target >= c0) & (target < c1)
    mask1 = (target >= c1) & (target < c2)
    idx0 = np.where(mask0)[0]
    idx1 = np.where(mask1)[0]
    t0n, t1n = len(idx0), len(idx1)
    b0 = max(1, -(-t0n // 128))
    b1 = max(1, -(-t1n // 128))
    t0c, t1c = b0 * 128, b1 * 128
    first_t = np.where(mask0, c0, np.where(mask1, c0 + 1, target))

    def chunks(a, k, dtype=BF16):  # [k*128, F] -> [k, 128, F]
        return np.ascontiguousarray(a.reshape(k, 128, a.shape[1])).astype(dtype)

    def padT(a, tcap):  # [T, F] -> [F, tcap]
        out = np.zeros((a.shape[1], tcap), f32)
        out[:, :a.shape[0]] = a.T
        return out

    # compacted tail inputs (same on every core)
    x08 = chunks(padT(w_in[idx0], t0c), K0, FP8)
    x18 = chunks(padT(w_in[idx1], t1c), K0, FP8)
    g0 = chunks(padT(t0w2[target[idx0] - c0], t0c), K0)
    g1 = chunks(padT(t1w2[target[idx1] - c1], t1c), K1)
    w1t0 = chunks(t0w1.T * WSCALE, K0, FP8)
    w1t1 = chunks(t1w1.T * WSCALE, K0, FP8)
    hwt8 = chunks(head_w.T * WSCALE, K0, FP8)
    hbias = (head_b[None, :] * WSCALE).astype(BF16)
    w2t0_full = (t0w2.T * WSCALE)                  # [1024, 8000]
    w2t1_full = (t1w2.T * WSCALE)                  # [256, 40000]

    gh_rows = head_w[first_t]                      # [N_TOK, 1024]
    bh = head_b[first_t]                           # [N_TOK]

    in_maps = []
    for c in range(N_CORES):
        sl = slice(c * TOK_PER_CORE, (c + 1) * TOK_PER_CORE)
        xt = np.zeros((KX * 128, TOK_PER_CORE), f32)
        xt[:D] = w_in[sl].T
        xt[D] = 1.0
        ghm = np.zeros((KX * 128, TOK_PER_CORE), f32)
        ghm[:D] = gh_rows[sl].T
        ghm[D] = bh[sl]
        in_maps.append({
            "xh8": chunks(xt[:D], K0, FP8),
            "hwt8": hwt8, "hbias": hbias,
            "xt": chunks(xt, KX),
            "gh": chunks(ghm, KX),
            "x08": x08, "x18": x18,
            "w1t0": w1t0, "w1t1": w1t1,
            "w2t0": chunks(w2t0_full[:, c * V0S:(c + 1) * V0S], K0, FP8),
            "w2t1": chunks(w2t1_full[:, c * V1S:(c + 1) * V1S], K1, FP8),
            "g0": g0, "g1": g1,
        })
    return in_maps, (b0, b1, t0n, t1n)


def _combine(results, cols, meta):
    b0, b1, t0n, t1n = meta
    t0c, t1c = b0 * 128, b1 * 128
    total = 0.0
    S0 = np.zeros((128, b0))
    S1 = np.zeros((128, b1))
    for c in range(N_CORES):
        S = results[c]["out_s"].astype(np.float64)
        Sh = np.zeros((128, N_BLK))
        for j, (k, b) in enumerate(cols):
            if k == "h":
                Sh[:, b] += S[:, j]
            elif k == "t0":
                S0[:, b] += S[:, j]
            else:
                S1[:, b] += S[:, j]
        llh = results[c]["out_ll"].astype(np.float64)[0, :TOK_PER_CORE]
        llh = llh.reshape(N_BLK, 128).T                  # [p, b]
        total += (np.log(Sh) - llh).sum()
    ll0 = results[0]["out_ll"].astype(np.float64)[0, TOK_PER_CORE:TOK_PER_CORE + t0c]
    ll1 = results[0]["out_ll"].astype(np.float64)[0, TOK_PER_CORE + t0c:TOK_PER_CORE + t0c + t1c]
    s0 = S0.T.reshape(-1)[:t0n]                          # token j = b*128 + p
    s1 = S1.T.reshape(-1)[:t1n]
    total += (np.log(s0) - ll0[:t0n]).sum()
    total += (np.log(s1) - ll1[:t1n]).sum()
    return np.float32(total / N_TOK)


def _run(inputs, trace=False):
    from concourse.bass_utils import run_bass_kernel_spmd

    in_maps, meta = _prep_inputs(**inputs)
    key = (meta[0], meta[1])
    if key not in _cache:
        _cache[key] = _build_nc(*key)
    nc, cols = _cache[key]
    res = run_bass_kernel_spmd(nc, in_maps, core_ids=list(range(N_CORES)), trace=trace)
    loss = _combine(res.results, cols, meta)
    return loss, res


def kernel(**inputs) -> np.ndarray:
    loss, _ = _run(inputs, trace=False)
    return loss
